# revision 16
# baseline (speedup 1.0000x reference)
"""Trainium2 Bass kernel for nn_AttenConv (sparse_attention).

Math (see reference):
  e_j = user_emb @ W ; e_k = item_emb @ W
  u_neigh = segsum_row(adj, e_k) ; i_neigh = segsum_col(adj, e_j)
  out = softmax(u_neigh @ i_neigh.T, axis=1) @ e_k @ W

Kernel formulation:
  A (dense [U,I], fp16) built on host from COO.  W commutes with segsum:
    u_neigh = (A @ item_emb) @ W ; i_neigh = (A.T @ user_emb) @ W
  ekW = item_emb @ (W @ W).
  Sharding: users row-sharded (u-side), items col-sharded (i-side segsum)
  with AllGather of i_neighT; attention fully user-sharded.
  Precision: segsum fp16 (fp32 psum), W-applies/dot in fp32r, atten fp16,
  per-row max from a bf16/fp16 pass-1 (max cancels in softmax).
"""
import sys
sys.path.insert(0, '/opt/trn_rl_repo')
import numpy as np

import concourse.bass as bass
import concourse.mybir as mybir
import concourse.tile as tile
from concourse import bacc, bass_utils
from concourse.masks import make_identity

F32 = mybir.dt.float32
F32R = mybir.dt.float32r
F16 = mybir.dt.float16
AX = mybir.AxisListType.X
AF = mybir.ActivationFunctionType

D = 128
P = 128


def build_nc(nu, ni, cores):
    UL = nu // cores          # users per core
    IL = ni // cores          # items per core (i-side segsum shard)
    UHW = min(512, UL)        # user window for pass1/2
    NUH = UL // UHW           # user windows per core
    NUB = UL // P             # 128-user blocks per core
    UBW = UHW // P            # 128-user blocks per window
    NIC = ni // P             # 128-item chunks
    NIB = ni // 512           # 512-item blocks (pass-1 rhs)
    NUC = nu // P             # 128-user chunks (i-side segsum contraction)
    ILW = min(512, IL)
    NIW = IL // ILW

    nc = bacc.Bacc("TRN2", debug=False, num_devices=cores)

    a_ut = nc.dram_tensor("a_ut", [ni, UL], F16, kind="ExternalInput")
    a_i = nc.dram_tensor("a_i", [nu, IL], F16, kind="ExternalInput")
    item16 = nc.dram_tensor("item16", [ni, D], F16, kind="ExternalInput")
    itemT16 = nc.dram_tensor("itemT16", [D, ni], F16, kind="ExternalInput")
    user16 = nc.dram_tensor("user16", [nu, D], F16, kind="ExternalInput")
    w_in = nc.dram_tensor("w", [D, D], F32, kind="ExternalInput")
    wt_in = nc.dram_tensor("wt", [D, D], F32, kind="ExternalInput")
    out_sl = nc.dram_tensor("out_sl", [UL, D], F32, kind="ExternalOutput")

    cc_in = nc.dram_tensor("cc_in", [D, IL], F32)
    cc_out = nc.dram_tensor("cc_out", [cores * D, IL], F32, addr_space="Shared")

    with tile.TileContext(nc) as tc:
        with (
            tc.tile_pool(name="const", bufs=1) as cpool,
            tc.tile_pool(name="big", bufs=1) as bigp,
            tc.tile_pool(name="astream", bufs=3) as apool,
            tc.tile_pool(name="work", bufs=1) as wpool,
            tc.tile_pool(name="negmp", bufs=2) as negmp,
            tc.tile_pool(name="mt", bufs=2) as mtpool,
            tc.tile_pool(name="at", bufs=3) as atpool,
            tc.tile_pool(name="ps", bufs=2, space="PSUM") as pspool,
            tc.tile_pool(name="ps_seg", bufs=1, space="PSUM") as ps_seg,
            tc.tile_pool(name="ps_out", bufs=1, space="PSUM") as ps_out,
        ):
            # ---------- constants ----------
            w_sb = cpool.tile([D, D], F32)
            wt_sb = cpool.tile([D, D], F32)
            nc.sync.dma_start(out=w_sb[:], in_=w_in[:])
            nc.sync.dma_start(out=wt_sb[:], in_=wt_in[:])
            ident = cpool.tile([P, P], F32)
            make_identity(nc, ident[:])
            ones1 = cpool.tile([1, P], F16)
            nc.gpsimd.memset(ones1[:], 1.0)

            user_sb = bigp.tile([P, nu], F16)
            nc.sync.dma_start(
                out=user_sb[:].rearrange("p (c d) -> p c d", d=D),
                in_=user16[:].rearrange("(c p) d -> p c d", p=P),
            )
            item_sb = bigp.tile([P, ni], F16)
            nc.sync.dma_start(
                out=item_sb[:].rearrange("p (c d) -> p c d", d=D),
                in_=item16[:].rearrange("(c p) d -> p c d", p=P),
            )
            itemT_sb = bigp.tile([P, ni], F16)
            nc.sync.dma_start(out=itemT_sb[:], in_=itemT16[:])

            # ---------- i-side segsum (start early: feeds AllGather) ----------
            # i_rawT[d, i_loc] = sum_u user[u, d] * A[u, i_loc]
            i_nT_loc = wpool.tile([D, IL], F32, tag="inTloc")
            GRPI = 4 if NUC % 4 == 0 else 1
            GRPU = 8 if NIC % 8 == 0 else 1
            ps_iw = [ps_seg.tile([D, ILW], F32, tag=f"segps{iw}", name=f"psiw{iw}")
                     for iw in range(NIW)]
            for ug in range(NUC // GRPI):
                a_sl = apool.tile([P, GRPI, IL], F16, tag="aslice")
                nc.sync.dma_start(
                    out=a_sl[:],
                    in_=a_i[ug * GRPI * P:(ug + 1) * GRPI * P, :].rearrange(
                        "(g p) i -> p g i", p=P),
                )
                for g in range(GRPI):
                    uch = ug * GRPI + g
                    for iw in range(NIW):
                        nc.tensor.matmul(
                            ps_iw[iw][:],
                            lhsT=user_sb[:, uch * P:(uch + 1) * P],
                            rhs=a_sl[:, g, iw * ILW:(iw + 1) * ILW],
                            start=(uch == 0), stop=(uch == NUC - 1),
                        )
            for iw in range(NIW):
                nc.vector.tensor_copy(
                    out=i_nT_loc[:, iw * ILW:(iw + 1) * ILW], in_=ps_iw[iw][:])
            # W-apply: i_final_T = W.T-contract: lhsT=w_r [K=d1, M=d2]
            i_fin = wpool.tile([D, IL], F32, tag="ifin")
            for iw in range(NIW):
                ps = ps_seg.tile([D, ILW], F32, tag=f"segps{iw}")
                nc.tensor.matmul(
                    ps[:], lhsT=w_sb[:],
                    rhs=i_nT_loc[:, iw * ILW:(iw + 1) * ILW],
                    start=True, stop=True,
                )
                nc.vector.tensor_copy(
                    out=i_fin[:, iw * ILW:(iw + 1) * ILW], in_=ps[:])
            nc.sync.dma_start(out=cc_in[:], in_=i_fin[:])
            nc.gpsimd.collective_compute(
                "AllGather", mybir.AluOpType.bypass,
                ins=[cc_in[:].opt()],
                outs=[cc_out[:].opt()],
                replica_groups=[list(range(cores))],
            )

            # ---------- W2 and ekW (+ones) ----------
            ps_w2 = ps_seg.tile([D, D], F32, tag="segps0")
            nc.tensor.matmul(ps_w2[:], lhsT=wt_sb[:], rhs=w_sb[:],
                             start=True, stop=True)
            w2_16 = cpool.tile([D, D], F16)
            nc.vector.tensor_copy(out=w2_16[:], in_=ps_w2[:])
            ekwo = bigp.tile([P, NIC * (D + 1)], F16)
            nc.gpsimd.memset(ekwo[:], 1.0)
            for c in range(NIC):
                ps = ps_seg.tile([P, D], F32, tag="segps0")
                nc.tensor.matmul(
                    ps[:], lhsT=itemT_sb[:, c * P:(c + 1) * P], rhs=w2_16[:],
                    start=True, stop=True)
                nc.scalar.copy(
                    out=ekwo[:, c * (D + 1):c * (D + 1) + D], in_=ps[:])

            # ---------- gather i_neighT from AllGather ----------
            i_hi = bigp.tile([D, ni], F16)
            i_lo = bigp.tile([D, ni], F16)
            for cblk in range(cores):
                itmp = wpool.tile([D, IL], F32, tag="itmp")
                nc.sync.dma_start(
                    out=itmp[:],
                    in_=cc_out[cblk * D:(cblk + 1) * D, :],
                )
                nc.vector.tensor_copy(
                    out=i_hi[:, cblk * IL:(cblk + 1) * IL], in_=itmp[:])
                nc.vector.tensor_tensor(
                    out=i_lo[:, cblk * IL:(cblk + 1) * IL], in0=itmp[:],
                    in1=i_hi[:, cblk * IL:(cblk + 1) * IL],
                    op=mybir.AluOpType.subtract)

            # ---------- per user-window: segsum-u, pass1, pass2 ----------
            negm_list = []
            uhw_list = []
            for uw in range(NUH):
                # ---- u-side segsum for this window
                ps_u = ps_seg.tile([D, UHW], F32, tag="segps1",
                                   name=f"psu{uw}")
                for ig in range(NIC // GRPU):
                    a_sl = apool.tile([P, GRPU, UHW], F16, tag="aslice",
                                      name=f"aslu{uw}_{ig}")
                    nc.sync.dma_start(
                        out=a_sl[:],
                        in_=a_ut[ig * GRPU * P:(ig + 1) * GRPU * P,
                                 uw * UHW:(uw + 1) * UHW].rearrange(
                            "(g p) u -> p g u", p=P),
                    )
                    for g in range(GRPU):
                        ich = ig * GRPU + g
                        nc.tensor.matmul(
                            ps_u[:],
                            lhsT=item_sb[:, ich * P:(ich + 1) * P],
                            rhs=a_sl[:, g, :],
                            start=(ich == 0), stop=(ich == NIC - 1),
                        )
                u_rawT = wpool.tile([D, UHW], F32, tag=f"uraw{uw}",
                                    name=f"uraw{uw}")
                nc.vector.tensor_copy(out=u_rawT[:], in_=ps_u[:])
                ps_w = ps_seg.tile([D, UHW], F32, tag="segps1",
                                   name=f"psw{uw}")
                nc.tensor.matmul(ps_w[:], lhsT=w_sb[:], rhs=u_rawT[:],
                                 start=True, stop=True)
                u_nTw = wpool.tile([D, UHW], F32, tag=f"unT{uw}",
                                   name=f"unT{uw}")
                nc.vector.tensor_copy(out=u_nTw[:], in_=ps_w[:])
                u_hi = wpool.tile([D, UHW], F16, tag=f"uhi{uw}",
                                  name=f"uhi{uw}")
                nc.vector.tensor_copy(out=u_hi[:], in_=u_nTw[:])
                u_lo = wpool.tile([D, UHW], F16, tag=f"ulo{uw}",
                                  name=f"ulo{uw}")
                nc.vector.tensor_tensor(out=u_lo[:], in0=u_nTw[:],
                                        in1=u_hi[:],
                                        op=mybir.AluOpType.subtract)
                uhw_list.append((u_hi, u_lo))

                # ---- pass 1: row max over items, per 128-user block
                negm = negmp.tile([1, UHW], F16, tag="negm",
                                  name=f"negm{uw}")
                negm_list.append(negm)
                for b in range(UBW):
                    mbig = mtpool.tile([P, ni], F16, tag="mbig")
                    for ib in range(NIB):
                        ps = pspool.tile([P, 512], F32, tag="dps")
                        nc.tensor.matmul(
                            ps[:],
                            lhsT=u_hi[:, b * P:(b + 1) * P],
                            rhs=i_hi[:, ib * 512:(ib + 1) * 512],
                            start=True, stop=True)
                        nc.scalar.copy(
                            out=mbig[:, ib * 512:(ib + 1) * 512], in_=ps[:])
                    # max tree on [P, ni] fp16
                    width = ni
                    while width > 512:
                        half = width // 2
                        nc.vector.tensor_tensor(
                            out=mbig[:, :half], in0=mbig[:, :half],
                            in1=mbig[:, half:width], op=mybir.AluOpType.max)
                        width = half
                    mcol = mtpool.tile([P, 1], F32, tag="mcol")
                    nc.vector.reduce_max(out=mcol[:], in_=mbig[:, :width], axis=AX)
                    ps_tr = ps_seg.tile([1, P], F32, tag="segps0")
                    nc.tensor.transpose(out=ps_tr[:], in_=mcol[:], identity=ident[:])
                    nc.vector.tensor_scalar_mul(
                        out=negm[:, b * P:(b + 1) * P], in0=ps_tr[:], scalar1=-1.0)

                # ---- pass 2
                ops = [ps_out.tile([P, D + 1], F32, tag=f"ops{b}",
                                   name=f"ops_{uw}_{b}")
                       for b in range(UBW)]
                for c in range(NIC):
                    ps = pspool.tile([P, UHW], F32, tag="dps")
                    usl = slice(0, UHW)
                    csl = slice(c * P, (c + 1) * P)
                    nc.tensor.matmul(
                        ps[:], lhsT=i_hi[:, csl], rhs=u_hi[:, usl],
                        start=True, stop=False)
                    nc.tensor.matmul(
                        ps[:], lhsT=i_hi[:, csl], rhs=u_lo[:, usl],
                        start=False, stop=False)
                    nc.tensor.matmul(
                        ps[:], lhsT=i_lo[:, csl], rhs=u_hi[:, usl],
                        start=False, stop=False)
                    nc.tensor.matmul(
                        ps[:], lhsT=ones1[:], rhs=negm[:],
                        start=False, stop=True)
                    at = atpool.tile([P, UHW], F16, tag="at")
                    nc.scalar.activation(at[:], ps[:], AF.Exp)
                    for b in range(UBW):
                        nc.tensor.matmul(
                            ops[b][:],
                            lhsT=at[:, b * P:(b + 1) * P],
                            rhs=ekwo[:, c * (D + 1):(c + 1) * (D + 1)],
                            start=(c == 0), stop=(c == NIC - 1))

                # ---- finalize: divide by Z, store
                for b in range(UBW):
                    ub = uw * UBW + b
                    src = ops[b][:]
                    rec = mtpool.tile([P, 1], F32, tag="rec")
                    nc.vector.reciprocal(out=rec[:], in_=src[:, D:D + 1])
                    o_sb = mtpool.tile([P, D], F32, tag="osb")
                    nc.vector.tensor_scalar_mul(
                        out=o_sb[:], in0=src[:, 0:D], scalar1=rec[:])
                    nc.sync.dma_start(
                        out=out_sl[ub * P:(ub + 1) * P, :], in_=o_sb[:])

    nc.compile()
    return nc


def _prep_inputs(user_emb, item_emb, W, adj_val, adj_row, adj_col, cores):
    nu, d = user_emb.shape
    ni = item_emb.shape[0]
    UL, IL = nu // cores, ni // cores
    A = np.zeros((nu, ni), np.float32)
    np.add.at(A, (adj_row, adj_col), adj_val)
    A16 = A.astype(np.float16)
    AT16 = np.ascontiguousarray(A16.T)
    item16 = item_emb.astype(np.float16)
    itemT16 = np.ascontiguousarray(item16.T)
    user16 = user_emb.astype(np.float16)
    W = np.ascontiguousarray(W, np.float32)
    WT = np.ascontiguousarray(W.T)
    in_maps = []
    for m in range(cores):
        in_maps.append({
            "a_ut": np.ascontiguousarray(AT16[:, m * UL:(m + 1) * UL]),
            "a_i": np.ascontiguousarray(A16[:, m * IL:(m + 1) * IL]),
            "item16": item16,
            "itemT16": itemT16,
            "user16": user16,
            "w": W,
            "wt": WT,
        })
    return in_maps


_NC_CACHE = {}


def _ensure_ntff_hook():
    """Register the axon NTFF profiling hook (image's antenv lacks it)."""
    import types
    if 'antenv.axon_hooks' not in sys.modules:
        mod = types.ModuleType('antenv.axon_hooks')
        _h = {'v': None}
        mod.set_axon_ntff_profile_hook = lambda h: _h.__setitem__('v', h)
        mod.get_axon_ntff_profile_hook = lambda: _h['v']
        sys.modules['antenv.axon_hooks'] = mod
        try:
            import antenv
            antenv.axon_hooks = mod
        except ImportError:
            pass
    mod = sys.modules['antenv.axon_hooks']
    if mod.get_axon_ntff_profile_hook() is None:
        try:
            sys.path.insert(0, '/root/.axon_site/trn_agent_boot')
            import trn_boot
            hook = trn_boot._ntff_profile_via_ctypes('/opt/axon/libaxon_pjrt.so')
            if hook is not None:
                mod.set_axon_ntff_profile_hook(hook)
        except Exception:
            pass
    bass_utils.upload_artifacts = lambda d: d


def kernel(user_emb, item_emb, W, adj_val, adj_row, adj_col,
           cores=8, _trace=False):
    user_emb = np.asarray(user_emb, np.float32)
    item_emb = np.asarray(item_emb, np.float32)
    W = np.asarray(W, np.float32)
    adj_val = np.asarray(adj_val, np.float32)
    adj_row = np.asarray(adj_row, np.int32)
    adj_col = np.asarray(adj_col, np.int32)

    nu, ni = user_emb.shape[0], item_emb.shape[0]
    key = (nu, ni, cores)
    if key not in _NC_CACHE:
        _NC_CACHE[key] = build_nc(nu, ni, cores)
    nc = _NC_CACHE[key]

    in_maps = _prep_inputs(user_emb, item_emb, W, adj_val, adj_row,
                           adj_col, cores)
    if _trace:
        _ensure_ntff_hook()
        try:
            res = bass_utils.run_bass_kernel_spmd(
                nc, in_maps, core_ids=list(range(cores)), trace=True)
        except Exception as e:
            print(f"trace run failed ({e!r}); retrying without trace",
                  flush=True)
            res = bass_utils.run_bass_kernel_spmd(
                nc, in_maps, core_ids=list(range(cores)), trace=False)
    else:
        res = bass_utils.run_bass_kernel_spmd(
            nc, in_maps, core_ids=list(range(cores)), trace=False)
    UL = nu // cores
    out = np.empty((nu, 128), np.float32)
    for m in range(cores):
        out[m * UL:(m + 1) * UL] = res.results[m]["out_sl"]
    if _trace:
        kernel._last_res = res
    return out


# revision 18
# speedup vs baseline: 1.0533x; 1.0533x over previous
"""Trainium2 Bass kernel for nn_AttenConv (sparse_attention).

Math (see reference):
  e_j = user_emb @ W ; e_k = item_emb @ W
  u_neigh = segsum_row(adj, e_k) ; i_neigh = segsum_col(adj, e_j)
  out = softmax(u_neigh @ i_neigh.T, axis=1) @ e_k @ W

Kernel formulation:
  A (dense [U,I], fp16) built on host from COO.  W commutes with segsum:
    u_neigh = (A @ item_emb) @ W ; i_neigh = (A.T @ user_emb) @ W
  ekW = item_emb @ (W @ W).
  Sharding: users row-sharded (u-side), items col-sharded (i-side segsum)
  with AllGather of i_neighT; attention fully user-sharded.
  Precision: segsum fp16 (fp32 psum), W-applies/dot in fp32r, atten fp16,
  per-row max from a bf16/fp16 pass-1 (max cancels in softmax).
"""
import sys
sys.path.insert(0, '/opt/trn_rl_repo')
import numpy as np

import concourse.bass as bass
import concourse.mybir as mybir
import concourse.tile as tile
from concourse import bacc, bass_utils
from concourse.masks import make_identity

F32 = mybir.dt.float32
F32R = mybir.dt.float32r
F16 = mybir.dt.float16
AX = mybir.AxisListType.X
AF = mybir.ActivationFunctionType

D = 128
P = 128


def build_nc(nu, ni, cores):
    UL = nu // cores          # users per core
    IL = ni // cores          # items per core (i-side segsum shard)
    UHW = min(512, UL)        # user window for pass1/2
    NUH = UL // UHW           # user windows per core
    NUB = UL // P             # 128-user blocks per core
    UBW = UHW // P            # 128-user blocks per window
    NIC = ni // P             # 128-item chunks
    NIB = ni // 512           # 512-item blocks (pass-1 rhs)
    NUC = nu // P             # 128-user chunks (i-side segsum contraction)
    ILW = min(512, IL)
    NIW = IL // ILW

    nc = bacc.Bacc("TRN2", debug=False, num_devices=cores)

    a_ut = nc.dram_tensor("a_ut", [ni, UL], F16, kind="ExternalInput")
    a_i = nc.dram_tensor("a_i", [nu, IL], F16, kind="ExternalInput")
    item16 = nc.dram_tensor("item16", [ni, D], F16, kind="ExternalInput")
    itemT16 = nc.dram_tensor("itemT16", [D, ni], F16, kind="ExternalInput")
    user16 = nc.dram_tensor("user16", [nu, D], F16, kind="ExternalInput")
    w_in = nc.dram_tensor("w", [D, D], F32, kind="ExternalInput")
    wt_in = nc.dram_tensor("wt", [D, D], F32, kind="ExternalInput")
    out_sl = nc.dram_tensor("out_sl", [UL, D], F32, kind="ExternalOutput")

    cc_in = nc.dram_tensor("cc_in", [D, IL], F32)
    cc_out = nc.dram_tensor("cc_out", [cores * D, IL], F32, addr_space="Shared")

    with tile.TileContext(nc) as tc:
        with (
            tc.tile_pool(name="const", bufs=1) as cpool,
            tc.tile_pool(name="big", bufs=1) as bigp,
            tc.tile_pool(name="astream", bufs=3) as apool,
            tc.tile_pool(name="work", bufs=1) as wpool,
            tc.tile_pool(name="negmp", bufs=2) as negmp,
            tc.tile_pool(name="mt", bufs=2) as mtpool,
            tc.tile_pool(name="at", bufs=3) as atpool,
            tc.tile_pool(name="ps", bufs=2, space="PSUM") as pspool,
            tc.tile_pool(name="ps_seg", bufs=1, space="PSUM") as ps_seg,
            tc.tile_pool(name="ps_out", bufs=1, space="PSUM") as ps_out,
        ):
            # ---------- constants ----------
            w_sb = cpool.tile([D, D], F32)
            wt_sb = cpool.tile([D, D], F32)
            nc.sync.dma_start(out=w_sb[:], in_=w_in[:])
            nc.sync.dma_start(out=wt_sb[:], in_=wt_in[:])
            ident = cpool.tile([P, P], F32)
            make_identity(nc, ident[:])
            ones1 = cpool.tile([1, P], F16)
            nc.gpsimd.memset(ones1[:], 1.0)

            user_sb = bigp.tile([P, nu], F16)
            nc.sync.dma_start(
                out=user_sb[:].rearrange("p (c d) -> p c d", d=D),
                in_=user16[:].rearrange("(c p) d -> p c d", p=P),
            )
            item_sb = bigp.tile([P, ni], F16)
            nc.sync.dma_start(
                out=item_sb[:].rearrange("p (c d) -> p c d", d=D),
                in_=item16[:].rearrange("(c p) d -> p c d", p=P),
            )
            itemT_sb = bigp.tile([P, ni], F16)
            nc.sync.dma_start(out=itemT_sb[:], in_=itemT16[:])

            # ---------- i-side segsum (start early: feeds AllGather) ----------
            # i_rawT[d, i_loc] = sum_u user[u, d] * A[u, i_loc]
            i_nT_loc = wpool.tile([D, IL], F32, tag="inTloc")
            GRP = 4 if NUC % 4 == 0 else 1
            ps_iw = [ps_seg.tile([D, ILW], F32, tag=f"segps{iw}", name=f"psiw{iw}")
                     for iw in range(NIW)]
            for ug in range(NUC // GRP):
                a_sl = apool.tile([P, GRP, IL], F16, tag="aslice")
                nc.sync.dma_start(
                    out=a_sl[:],
                    in_=a_i[ug * GRP * P:(ug + 1) * GRP * P, :].rearrange(
                        "(g p) i -> p g i", p=P),
                )
                for g in range(GRP):
                    uch = ug * GRP + g
                    for iw in range(NIW):
                        nc.tensor.matmul(
                            ps_iw[iw][:],
                            lhsT=user_sb[:, uch * P:(uch + 1) * P],
                            rhs=a_sl[:, g, iw * ILW:(iw + 1) * ILW],
                            start=(uch == 0), stop=(uch == NUC - 1),
                        )
            for iw in range(NIW):
                nc.vector.tensor_copy(
                    out=i_nT_loc[:, iw * ILW:(iw + 1) * ILW], in_=ps_iw[iw][:])
            # W-apply: i_final_T = W.T-contract: lhsT=w_r [K=d1, M=d2]
            i_fin = wpool.tile([D, IL], F32, tag="ifin")
            for iw in range(NIW):
                ps = ps_seg.tile([D, ILW], F32, tag=f"segps{iw}")
                nc.tensor.matmul(
                    ps[:], lhsT=w_sb[:],
                    rhs=i_nT_loc[:, iw * ILW:(iw + 1) * ILW],
                    start=True, stop=True,
                )
                nc.vector.tensor_copy(
                    out=i_fin[:, iw * ILW:(iw + 1) * ILW], in_=ps[:])
            nc.sync.dma_start(out=cc_in[:], in_=i_fin[:])
            nc.gpsimd.collective_compute(
                "AllGather", mybir.AluOpType.bypass,
                ins=[cc_in[:].opt()],
                outs=[cc_out[:].opt()],
                replica_groups=[list(range(cores))],
            )

            # ---------- u-side segsum ----------
            u_rawT = wpool.tile([D, UL], F32, tag="urawT")
            ps_uw = [ps_seg.tile([D, UHW], F32, tag=f"segps{uw}", name=f"psuw{uw}")
                     for uw in range(NUH)]
            for ig in range(NIC // GRP):
                a_sl = apool.tile([P, GRP, UL], F16, tag="aslice")
                nc.sync.dma_start(
                    out=a_sl[:],
                    in_=a_ut[ig * GRP * P:(ig + 1) * GRP * P, :].rearrange(
                        "(g p) u -> p g u", p=P),
                )
                for g in range(GRP):
                    ich = ig * GRP + g
                    for uw in range(NUH):
                        nc.tensor.matmul(
                            ps_uw[uw][:],
                            lhsT=item_sb[:, ich * P:(ich + 1) * P],
                            rhs=a_sl[:, g, uw * UHW:(uw + 1) * UHW],
                            start=(ich == 0), stop=(ich == NIC - 1),
                        )
            for uw in range(NUH):
                nc.vector.tensor_copy(
                    out=u_rawT[:, uw * UHW:(uw + 1) * UHW], in_=ps_uw[uw][:])
            u_nT = wpool.tile([D, UL], F32, tag="unT")
            for uw in range(NUH):
                ps = ps_seg.tile([D, UHW], F32, tag=f"segps{uw}")
                nc.tensor.matmul(
                    ps[:], lhsT=w_sb[:],
                    rhs=u_rawT[:, uw * UHW:(uw + 1) * UHW],
                    start=True, stop=True,
                )
                nc.vector.tensor_copy(
                    out=u_nT[:, uw * UHW:(uw + 1) * UHW], in_=ps[:])
            u_hi = wpool.tile([D, UL], F16, tag="uhi")
            nc.vector.tensor_copy(out=u_hi[:], in_=u_nT[:])
            u_lo = wpool.tile([D, UL], F16, tag="ulo")
            nc.vector.tensor_tensor(out=u_lo[:], in0=u_nT[:], in1=u_hi[:],
                                    op=mybir.AluOpType.subtract)

            # ---------- W2 and ekW (+ones) ----------
            ps_w2 = ps_seg.tile([D, D], F32, tag="segps0")
            nc.tensor.matmul(ps_w2[:], lhsT=wt_sb[:], rhs=w_sb[:],
                             start=True, stop=True)
            w2_16 = cpool.tile([D, D], F16)
            nc.vector.tensor_copy(out=w2_16[:], in_=ps_w2[:])
            ekwo = bigp.tile([P, NIC * (D + 1)], F16)
            nc.gpsimd.memset(ekwo[:], 1.0)
            for c in range(NIC):
                ps = ps_seg.tile([P, D], F32, tag="segps0")
                nc.tensor.matmul(
                    ps[:], lhsT=itemT_sb[:, c * P:(c + 1) * P], rhs=w2_16[:],
                    start=True, stop=True)
                nc.scalar.copy(
                    out=ekwo[:, c * (D + 1):c * (D + 1) + D], in_=ps[:])

            # ---------- gather i_neighT from AllGather ----------
            i_hi = bigp.tile([D, ni], F16)
            i_lo = bigp.tile([D, ni], F16)
            for cblk in range(cores):
                itmp = wpool.tile([D, IL], F32, tag="itmp")
                nc.sync.dma_start(
                    out=itmp[:],
                    in_=cc_out[cblk * D:(cblk + 1) * D, :],
                )
                nc.vector.tensor_copy(
                    out=i_hi[:, cblk * IL:(cblk + 1) * IL], in_=itmp[:])
                nc.vector.tensor_tensor(
                    out=i_lo[:, cblk * IL:(cblk + 1) * IL], in0=itmp[:],
                    in1=i_hi[:, cblk * IL:(cblk + 1) * IL],
                    op=mybir.AluOpType.subtract)

            # ---------- pass1 (max) for all user windows ----------
            negm_list = []
            for uw in range(NUH):
                # ---- pass 1: row max over items, per 128-user block
                negm = negmp.tile([1, UHW], F16, tag="negm",
                                  name=f"negm{uw}")
                negm_list.append(negm)
                for b in range(UBW):
                    ub = uw * UBW + b
                    mbig = mtpool.tile([P, ni], F16, tag="mbig")
                    for ib in range(NIB):
                        ps = pspool.tile([P, 512], F32, tag="dps")
                        nc.tensor.matmul(
                            ps[:],
                            lhsT=u_hi[:, ub * P:(ub + 1) * P],
                            rhs=i_hi[:, ib * 512:(ib + 1) * 512],
                            start=True, stop=True)
                        nc.scalar.copy(
                            out=mbig[:, ib * 512:(ib + 1) * 512], in_=ps[:])
                    # max tree on [P, ni] fp16
                    width = ni
                    while width > 512:
                        half = width // 2
                        nc.vector.tensor_tensor(
                            out=mbig[:, :half], in0=mbig[:, :half],
                            in1=mbig[:, half:width], op=mybir.AluOpType.max)
                        width = half
                    mcol = mtpool.tile([P, 1], F32, tag="mcol")
                    nc.vector.reduce_max(out=mcol[:], in_=mbig[:, :width], axis=AX)
                    ps_tr = ps_seg.tile([1, P], F32, tag="segps0")
                    nc.tensor.transpose(out=ps_tr[:], in_=mcol[:], identity=ident[:])
                    nc.vector.tensor_scalar_mul(
                        out=negm[:, b * P:(b + 1) * P], in0=ps_tr[:], scalar1=-1.0)

            # ---------- pass2 for all user windows ----------
            for uw in range(NUH):
                negm = negm_list[uw]
                ops = [ps_out.tile([P, D + 1], F32, tag=f"ops{b}",
                                   name=f"ops_{uw}_{b}")
                       for b in range(UBW)]
                for c in range(NIC):
                    ps = pspool.tile([P, UHW], F32, tag="dps")
                    usl = slice(uw * UHW, (uw + 1) * UHW)
                    csl = slice(c * P, (c + 1) * P)
                    nc.tensor.matmul(
                        ps[:], lhsT=i_hi[:, csl], rhs=u_hi[:, usl],
                        start=True, stop=False)
                    nc.tensor.matmul(
                        ps[:], lhsT=i_hi[:, csl], rhs=u_lo[:, usl],
                        start=False, stop=False)
                    nc.tensor.matmul(
                        ps[:], lhsT=i_lo[:, csl], rhs=u_hi[:, usl],
                        start=False, stop=False)
                    nc.tensor.matmul(
                        ps[:], lhsT=ones1[:], rhs=negm[:],
                        start=False, stop=True)
                    at = atpool.tile([P, UHW], F16, tag="at")
                    nc.scalar.activation(at[:], ps[:], AF.Exp)
                    for b in range(UBW):
                        nc.tensor.matmul(
                            ops[b][:],
                            lhsT=at[:, b * P:(b + 1) * P],
                            rhs=ekwo[:, c * (D + 1):(c + 1) * (D + 1)],
                            start=(c == 0), stop=(c == NIC - 1))

                # ---- finalize: divide by Z, store
                for b in range(UBW):
                    ub = uw * UBW + b
                    src = ops[b][:]
                    rec = mtpool.tile([P, 1], F32, tag="rec")
                    nc.vector.reciprocal(out=rec[:], in_=src[:, D:D + 1])
                    o_sb = mtpool.tile([P, D], F32, tag="osb")
                    nc.vector.tensor_scalar_mul(
                        out=o_sb[:], in0=src[:, 0:D], scalar1=rec[:])
                    nc.sync.dma_start(
                        out=out_sl[ub * P:(ub + 1) * P, :], in_=o_sb[:])

    nc.compile()
    return nc


def _prep_inputs(user_emb, item_emb, W, adj_val, adj_row, adj_col, cores):
    nu, d = user_emb.shape
    ni = item_emb.shape[0]
    UL, IL = nu // cores, ni // cores
    A = np.zeros((nu, ni), np.float32)
    np.add.at(A, (adj_row, adj_col), adj_val)
    A16 = A.astype(np.float16)
    AT16 = np.ascontiguousarray(A16.T)
    item16 = item_emb.astype(np.float16)
    itemT16 = np.ascontiguousarray(item16.T)
    user16 = user_emb.astype(np.float16)
    W = np.ascontiguousarray(W, np.float32)
    WT = np.ascontiguousarray(W.T)
    in_maps = []
    for m in range(cores):
        in_maps.append({
            "a_ut": np.ascontiguousarray(AT16[:, m * UL:(m + 1) * UL]),
            "a_i": np.ascontiguousarray(A16[:, m * IL:(m + 1) * IL]),
            "item16": item16,
            "itemT16": itemT16,
            "user16": user16,
            "w": W,
            "wt": WT,
        })
    return in_maps


_NC_CACHE = {}


def _ensure_ntff_hook():
    """Register the axon NTFF profiling hook (image's antenv lacks it)."""
    import types
    if 'antenv.axon_hooks' not in sys.modules:
        mod = types.ModuleType('antenv.axon_hooks')
        _h = {'v': None}
        mod.set_axon_ntff_profile_hook = lambda h: _h.__setitem__('v', h)
        mod.get_axon_ntff_profile_hook = lambda: _h['v']
        sys.modules['antenv.axon_hooks'] = mod
        try:
            import antenv
            antenv.axon_hooks = mod
        except ImportError:
            pass
    mod = sys.modules['antenv.axon_hooks']
    if mod.get_axon_ntff_profile_hook() is None:
        try:
            sys.path.insert(0, '/root/.axon_site/trn_agent_boot')
            import trn_boot
            hook = trn_boot._ntff_profile_via_ctypes('/opt/axon/libaxon_pjrt.so')
            if hook is not None:
                mod.set_axon_ntff_profile_hook(hook)
        except Exception:
            pass
    bass_utils.upload_artifacts = lambda d: d


def kernel(user_emb, item_emb, W, adj_val, adj_row, adj_col,
           cores=8, _trace=False):
    user_emb = np.asarray(user_emb, np.float32)
    item_emb = np.asarray(item_emb, np.float32)
    W = np.asarray(W, np.float32)
    adj_val = np.asarray(adj_val, np.float32)
    adj_row = np.asarray(adj_row, np.int32)
    adj_col = np.asarray(adj_col, np.int32)

    nu, ni = user_emb.shape[0], item_emb.shape[0]
    key = (nu, ni, cores)
    if key not in _NC_CACHE:
        _NC_CACHE[key] = build_nc(nu, ni, cores)
    nc = _NC_CACHE[key]

    in_maps = _prep_inputs(user_emb, item_emb, W, adj_val, adj_row,
                           adj_col, cores)
    if _trace:
        _ensure_ntff_hook()
        try:
            res = bass_utils.run_bass_kernel_spmd(
                nc, in_maps, core_ids=list(range(cores)), trace=True)
        except Exception as e:
            print(f"trace run failed ({e!r}); retrying without trace",
                  flush=True)
            res = bass_utils.run_bass_kernel_spmd(
                nc, in_maps, core_ids=list(range(cores)), trace=False)
    else:
        res = bass_utils.run_bass_kernel_spmd(
            nc, in_maps, core_ids=list(range(cores)), trace=False)
    UL = nu // cores
    out = np.empty((nu, 128), np.float32)
    for m in range(cores):
        out[m * UL:(m + 1) * UL] = res.results[m]["out_sl"]
    if _trace:
        kernel._last_res = res
    return out
